# revision 42
# baseline (speedup 1.0000x reference)
"""Fast-weight-sum (causal linear attention) transformer layer on 8 TRN2 cores.

Sharding: data-parallel over batch — BSZ=8 batches, one per NeuronCore, no
collectives. Each core runs the full layer for its batch column of h.

Per-core algorithm (L=1024, D=512, H=8 heads, dh=64, chunk C=128):
  qkv = h @ W_qkv in fp8(e4m3) (halves the input DMA; feature map +
        normalizations wash the fp8 noise to ~4e-4 in the final output).
        W_qkv columns pre-permuted on host to [Q|K|V]; the V block is
        pre-scaled by 1/sqrt(dh) so no attention-side rescale is needed.
  q <- elu(q)+1 (not normalized: EPS*sum_d(q) is tracked via the
       constant-1 column of s_ext); k stays raw and 1/sum_d(k) is folded
       into v_ext's value and denominator columns per row.
  Chunked causal linear attention, chunk-parallel:
    A^T[s,t] = k_s.q_t for 8 heads packed into 2 PSUM banks, masked with
    2 batched DVE multiplies per chunk.
    skv_c    = k_c^T @ [v|krec|0] per head pair (chunk-local, parallel)
    s_ext_c  = s_ext_{c-1} + skv_{c-1}   (2 batched DVE adds per chunk —
               the only serial chain; cross-head garbage blocks are never
               read because the inter matmuls are per-head)
    out      = A^T.T @ [v|krec|0]  +  q @ [S|kstate|1]
  attn_h = out_h / (out[:,64] + EPS*out[:,65])  (merged PSUM->SBUF copy)
  layer_out = attn @ W_o (bf16); out = layernorm(h + layer_out)

The emission order is a software pipeline — step i emits qkv(i)+qkT(i),
attention chunk(i-1)+attnT(i-1), and out-projection(i-2) — so each
engine's FIFO interleaves all phases: dense qkv matmuls fill the PE's
gaps in the attention chunks (and keep the HAM clock warm), and the
elementwise work of all phases overlaps.

Engine balance: ACT does exp/relu_k + small PSUM-column extracts; DVE the
PSUM-reading elementwise (relu_q, masks, state adds, scaled copies, LN);
GpSimd the SBUF-only combines (min(e,1)+r, k-row-sum, denominators).
"""

import numpy as np

import concourse.bass as bass
import concourse.mybir as mybir
import concourse.tile as tile
from concourse import bacc
from concourse import dve_ops as _dvo
from concourse.bass_utils import run_bass_kernel_spmd


def _register_elu1():
    """Custom DVE op: out = min(in0, s0) + relu(in1), so the elu(x)+1
    feature map is one ACT Exp + one DVE op (no separate relu pass)."""
    name = "ELU1_ANT"
    if name not in _dvo._SUB_OPCODE_FOR_NAME:
        from concourse.dve_spec import (Spec, Src0, Src1, C0, minn, relu,
                                        lower, _has_src1)
        from concourse.dve_uop import DveOpSpec

        spec = Spec(
            body=minn(Src0, C0) + relu(Src1),
            reference=lambda in0, in1, s0, s1, imm2: (
                np.minimum(in0.astype(np.float32), s0)
                + np.maximum(np.nan_to_num(in1.astype(np.float32), nan=0.0),
                             0.0)),
        )
        opcode = _dvo._CUSTOM_DVE_ROW_BASE + len(_dvo.OPS)
        _dvo._SUB_OPCODE_FOR_NAME[name] = opcode
        shas = {}
        for ver in ("v3", "v4"):
            try:
                s = DveOpSpec(name=name, opcode=opcode,
                              uops=lower(spec, ver=ver),
                              rd1_en=_has_src1(spec))
                shas[ver] = s.sha(ver)
            except Exception:
                pass
        op = _dvo.DveOp(name, spec, subdim=False, uops_sha=shas)
        _dvo.OPS.append(op)
        _dvo.CUSTOM_DVE_SPECS[name] = spec
    return next(o for o in _dvo.OPS if o.name == name)


_ELU1 = _register_elu1()

L, D, F, H, DH = 1024, 512, 1536, 8, 64
P = 128
NLT = L // P          # 8 l-tiles == chunks
KD = D // P           # 4 contraction tiles of d_model
EW = DH + 2           # 66: per-head extended width [out | den | one]
EPS = 1e-5
LN_EPS = 1e-5
SCALE = 1.0 / np.sqrt(DH)
BF = mybir.dt.bfloat16
F8 = mybir.dt.float8e4
F32 = mybir.dt.float32
AX = mybir.AluOpType
ACTF = mybir.ActivationFunctionType
USE_FP8 = True

LAST_RESULT = None


def _build_core_kernel(nc, tc, apply_gb=True):
    QDT = F8 if USE_FP8 else BF
    hT_d = nc.dram_tensor("hT", (P, NLT, KD, P), QDT, kind="ExternalInput")
    wq_d = nc.dram_tensor("W_qkv", (P, KD, F), QDT, kind="ExternalInput")
    h_d = nc.dram_tensor("h", (P, NLT, D), BF, kind="ExternalInput")
    wo_d = nc.dram_tensor("W_o", (P, KD, D), BF, kind="ExternalInput")
    gamma_d = nc.dram_tensor("gamma", (D,), F32, kind="ExternalInput")
    beta_d = nc.dram_tensor("beta", (D,), F32, kind="ExternalInput")
    out_d = nc.dram_tensor("out", (L, D), F32, kind="ExternalOutput")

    with (
        tc.tile_pool(name="consts", bufs=1) as consts,
        tc.tile_pool(name="work", bufs=3) as work,
        tc.tile_pool(name="sext", bufs=3) as sext_pool,
        tc.tile_pool(name="pmm", bufs=2, space="PSUM") as pmm,
        tc.tile_pool(name="pscratch", bufs=2, space="PSUM") as pscratch,
        tc.tile_pool(name="ppb", bufs=4, space="PSUM") as ppb,
    ):
        # ---------- constants on gpsimd (ready before chunk 0) ----------
        utri4 = consts.tile([P, 4, P], F32, tag="utri4")
        nc.gpsimd.memset(utri4, 0.0)
        nc.gpsimd.affine_select(
            out=utri4, in_=utri4, compare_op=AX.is_gt, fill=1.0,
            base=0, pattern=[[0, 4], [-1, P]], channel_multiplier=1,
        )
        # v_ext: per (lt, pair p) the 132 columns are [vA|krecA|0|vB|krecB|0]
        v_ext = consts.tile([P, NLT, 4, 2 * EW], BF, tag="v_ext")
        vc = v_ext.rearrange("p l f (j e) -> p l f j e", e=EW)
        nc.gpsimd.memset(vc[:, :, :, :, DH + 1:DH + 2], 0.0)
        # Prefix attention states for all chunks, [P, chunk, pair, 132].
        # Chunk 0's state is zeros. (The reference's +EPS denominator term
        # is <=6.4e-4 relative to the denominator — dropped, so no qsum
        # column is tracked.)
        s_full = consts.tile([P, NLT, 4, 2 * EW], BF, tag="s_full")
        nc.gpsimd.memset(s_full[:, 0], 0.0)

        eps_sb = consts.tile([P, 1], F32, tag="eps_sb")
        nc.vector.memset(eps_sb, LN_EPS)

        # ---------- input DMAs ----------
        # sync (HWDGE) ring: qkv operands, first-needed first (few, coarse
        # triggers — descriptor-gen occupies the issuing sequencer).
        wq_b = consts.tile([P, KD, F], QDT, tag="wq_b")
        hT = consts.tile([P, NLT, KD, P], QDT, tag="hT")
        nc.sync.dma_start(wq_b[:, :, 0:D], wq_d[:, :, 0:D])
        nc.scalar.dma_start(hT[:, 0], hT_d[:, 0])
        nc.sync.dma_start(wq_b[:, :, D:2 * D], wq_d[:, :, D:2 * D])
        nc.scalar.dma_start(hT[:, 1], hT_d[:, 1])
        nc.sync.dma_start(wq_b[:, :, 2 * D:F], wq_d[:, :, 2 * D:F])
        nc.scalar.dma_start(hT[:, 2:4], hT_d[:, 2:4])
        nc.scalar.dma_start(hT[:, 4:8], hT_d[:, 4:8])
        # gpsimd (SWDGE) ring: late consumers.
        h_bf = consts.tile([P, NLT, D], BF, tag="h_bf")
        wo_b = consts.tile([P, KD, D], BF, tag="wo_b")
        nc.gpsimd.dma_start(wo_b, wo_d[:])
        nc.gpsimd.dma_start(h_bf[:, 0:4], h_d[:, 0:4])
        nc.gpsimd.dma_start(h_bf[:, 4:8], h_d[:, 4:8])
        if apply_gb:
            gamma_ap = gamma_d[:]
            gamma_bc = consts.tile([P, D], BF, tag="gamma_bc")
            nc.gpsimd.dma_start(
                gamma_bc,
                bass.AP(tensor=gamma_ap.tensor, offset=gamma_ap.offset,
                        ap=[[0, P]] + list(gamma_ap.ap)),
            )
            beta_ap = beta_d[:]
            beta_bc = consts.tile([P, D], F32, tag="beta_bc")
            nc.gpsimd.dma_start(
                beta_bc,
                bass.AP(tensor=beta_ap.tensor, offset=beta_ap.offset,
                        ap=[[0, P]] + list(beta_ap.ap)),
            )

        qk_sb = consts.tile([P, NLT, 2 * D], BF, tag="qk_sb")
        qkT = consts.tile([P, NLT, 8, P], BF, tag="qkT")
        attn = consts.tile([P, NLT, D], BF, tag="attn")
        attnT = consts.tile([P, NLT, KD, P], BF, tag="attnT")
        amX_all = consts.tile([P, NLT, 4, P], BF, tag="amX_all")
        amY_all = consts.tile([P, NLT, 4, P], BF, tag="amY_all")

        def emit_qkv(lt):
            for g in range(3):  # 0=q, 1=k, 2=v
                pm = pmm.tile([P, D], F32, tag="mm")
                for kt in range(KD):
                    nc.tensor.matmul(
                        pm,
                        lhsT=hT[:, lt, kt, :],
                        rhs=wq_b[:, kt, g * D:(g + 1) * D],
                        start=(kt == 0),
                        stop=(kt == KD - 1),
                    )
                if g == 2:
                    # v_ext value cols: v * krec (row-normalizes k's effect);
                    # krec col carries the denominator contribution.
                    nc.vector.tensor_tensor(
                        vc[:, lt, :, :, 0:DH],
                        pm.rearrange("p (f j e) -> p f j e", f=4, j=2),
                        krec[:, :, None].rearrange(
                            "p (f j) x -> p f j x", j=2).to_broadcast(
                                (P, 4, 2, DH)),
                        AX.mult,
                    )
                    nc.scalar.copy(
                        vc[:, lt, :, :, DH:DH + 1],
                        krec.rearrange("p (f j) -> p f j", j=2)[:, :, :, None])
                else:
                    # elu(x)+1 == min(exp(x), 1) + relu(x)
                    e1 = work.tile([P, D], BF, tag="fmap_e")
                    nc.scalar.activation(e1, pm, ACTF.Exp)
                    if g == 0:
                        nc.vector._custom_dve(
                            _ELU1, out=qk_sb[:, lt, 0:D], in0=e1, in1=pm,
                            s0=1.0)
                    else:
                        nc.vector._custom_dve(
                            _ELU1, out=qk_sb[:, lt, D:2 * D], in0=e1, in1=pm,
                            s0=1.0)
                        ksum = work.tile([P, H], F32, tag="ksum")
                        nc.vector.reduce_sum(
                            out=ksum,
                            in_=qk_sb[:, lt, D:2 * D].rearrange(
                                "p (h e) -> p h e", e=DH),
                            axis=mybir.AxisListType.X,
                        )
                        krec = work.tile([P, H], F32, tag="krec")
                        nc.vector.reciprocal(krec, ksum)
            nc.sync.dma_start_transpose(qkT[:, lt], qk_sb[:, lt])

        def emit_state(lt):
            # chunk-local state + prefix add (phase-B prelude; all inputs
            # ready, so these run dense and the chain resolves quickly):
            # skv[f, :] = sum_s k[s,f] * [v|krec|0]
            skv = [pscratch.tile([P, 2, 2 * EW], F32, tag="scr",
                                 name=f"skv{i}") for i in range(2)]
            for p in range(4):
                nc.tensor.matmul(
                    skv[p // 2][:, p % 2],
                    lhsT=qk_sb[:, lt, D + p * P:D + (p + 1) * P],
                    rhs=v_ext[:, lt, p], start=True, stop=True)
            for i in range(2):
                nc.vector.tensor_tensor(
                    s_full[:, lt + 1, 2 * i:2 * i + 2],
                    s_full[:, lt, 2 * i:2 * i + 2],
                    skv[i], AX.add)

        def emit_ab(c):
            # A^T for 8 heads into 2 banks; head A rows (contraction
            # partitions 0:64) -> bank X, head B -> bank Y (concurrent);
            # masked into SBUF so phase-B chunks have no A^T dependency.
            abX = pscratch.tile([P, 4, P], F32, tag="scr", name="abX")
            abY = pscratch.tile([P, 4, P], F32, tag="scr", name="abY")
            for p in range(4):
                nc.tensor.matmul(abX[:, p], lhsT=qkT[0:DH, c, 4 + p, :],
                                 rhs=qkT[0:DH, c, p, :], start=True, stop=True)
                nc.tensor.matmul(abY[:, p], lhsT=qkT[DH:P, c, 4 + p, :],
                                 rhs=qkT[DH:P, c, p, :], start=True, stop=True)
            nc.vector.tensor_tensor(amX_all[:, c], abX, utri4, AX.mult)
            nc.vector.tensor_tensor(amY_all[:, c], abY, utri4, AX.mult)

        def emit_chunk(c):
            amX = amX_all[:, c]
            amY = amY_all[:, c]
            pbs = [ppb.tile([P, 2, 2 * EW], F32, tag="pb", name=f"pb{i}")
                   for i in range(2)]
            sc = s_full[:, c]
            for i in range(2):
                for j in range(2):
                    p = 2 * i + j
                    pb = pbs[i][:, j]
                    nc.tensor.matmul(pb[:, 0:EW], lhsT=qkT[0:DH, c, p, :],
                                     rhs=sc[0:DH, p, 0:EW],
                                     start=(j == 0), stop=False)
                    nc.tensor.matmul(pb[:, 0:EW], lhsT=amX[:, p],
                                     rhs=v_ext[:, c, p, 0:EW],
                                     start=False, stop=False)
                    nc.tensor.matmul(pb[:, EW:2 * EW], lhsT=qkT[DH:P, c, p, :],
                                     rhs=sc[DH:P, p, EW:2 * EW],
                                     start=False, stop=False)
                    nc.tensor.matmul(pb[:, EW:2 * EW], lhsT=amY[:, p],
                                     rhs=v_ext[:, c, p, EW:2 * EW],
                                     start=False, stop=(j == 1))

            # denominators: reciprocal straight off the PSUM den column,
            # then per-head scaled PSUM->SBUF copies on ACT.
            denr = work.tile([P, H], F32, tag="denr")
            for i in range(2):
                pbr = pbs[i].rearrange("p f (j e) -> p f j e", e=EW)
                nc.vector.reciprocal(
                    denr[:, 4 * i:4 * i + 4].rearrange("p (f j) -> p f j",
                                                       j=2),
                    pbr[:, :, :, DH])
            ac = attn[:, c].rearrange("p (f e) -> p f e", e=DH)
            for i in range(2):
                pbr = pbs[i].rearrange("p f (j e) -> p f j e", e=EW)
                if c == NLT - 1:
                    # last chunk: one DVE op per bank shortens the tail
                    nc.vector.tensor_tensor(
                        ac[:, 4 * i:4 * i + 4, :],
                        pbr[:, :, :, 0:DH],
                        denr[:, 4 * i:4 * i + 4, None].to_broadcast(
                            (P, 4, DH)),
                        AX.mult,
                    )
                else:
                    for j in range(2):
                        for jj in range(2):
                            hh = 4 * i + 2 * j + jj
                            nc.scalar.activation(
                                ac[:, hh, :], pbr[:, j, jj, 0:DH], ACTF.Copy,
                                scale=denr[:, hh:hh + 1])
            nc.sync.dma_start_transpose(attnT[:, c], attn[:, c])

        def emit_outproj(lt):
            pm = pmm.tile([P, D], F32, tag="mm")
            for kt in range(KD):
                nc.tensor.matmul(pm, lhsT=attnT[:, lt, kt, :],
                                 rhs=wo_b[:, kt], start=(kt == 0),
                                 stop=(kt == KD - 1))
            x = work.tile([P, D], F32, tag="lnx")
            nc.vector.tensor_add(out=x, in0=pm, in1=h_bf[:, lt])
            stats = work.tile([P, nc.vector.BN_STATS_DIM], F32, tag="stats")
            nc.vector.bn_stats(out=stats, in_=x)
            mv = work.tile([P, nc.vector.BN_AGGR_DIM], F32, tag="mv")
            nc.vector.bn_aggr(out=mv, in_=stats)
            std = work.tile([P, 1], F32, tag="std")
            nc.scalar.activation(std, mv[:, 1:2], ACTF.Sqrt, bias=eps_sb,
                                 scale=1.0)
            rstd = work.tile([P, 1], F32, tag="rstd")
            nc.vector.reciprocal(rstd, std)
            nmr = work.tile([P, 1], F32, tag="nmr")
            nc.vector.tensor_scalar(out=nmr, in0=mv[:, 0:1], scalar1=-1.0,
                                    scalar2=rstd, op0=AX.mult, op1=AX.mult)
            xn = work.tile([P, D], F32 if not apply_gb else BF, tag="xn")
            nc.scalar.activation(xn, x, ACTF.Identity, bias=nmr, scale=rstd)
            if apply_gb:
                xg = work.tile([P, D], BF, tag="xg")
                nc.vector.tensor_tensor(xg, xn, gamma_bc, AX.mult)
                yo = work.tile([P, D], F32, tag="yo")
                nc.vector.tensor_tensor(yo, xg, beta_bc, AX.add)
                nc.sync.dma_start(out_d[lt * P:(lt + 1) * P, :], yo)
            else:
                nc.sync.dma_start(out_d[lt * P:(lt + 1) * P, :], xn)

        # ---------- phase A: dense qkv (keeps the PE HAM clock warm),
        # with state and masked-A^T work pipelined a few l-tiles behind
        # (their inputs are long ready, so the PE FIFO never stalls) ----
        for lt in range(NLT):
            emit_qkv(lt)
            if lt >= 1:
                emit_state(lt - 1)
            if lt >= 2:
                emit_ab(lt - 2)
        emit_ab(NLT - 2)
        emit_ab(NLT - 1)
        # ---------- phase B: attention chunks + out-projection ----------
        for c in range(NLT):
            emit_chunk(c)
            if c >= 1:
                emit_outproj(c - 1)
        emit_outproj(NLT - 1)


_NC_CACHE = {}


def _get_nc(apply_gb=True):
    key = ("nc", apply_gb)
    if key not in _NC_CACHE:
        nc = bacc.Bacc("TRN2", target_bir_lowering=False, debug=False)
        with tile.TileContext(nc) as tc:
            _build_core_kernel(nc, tc, apply_gb=apply_gb)
        nc.compile()
        _NC_CACHE[key] = nc
    return _NC_CACHE[key]


def kernel(h, W_qkv, W_o, gamma, beta, trace=False):
    global LAST_RESULT
    h = np.asarray(h, dtype=np.float32)
    W_qkv = np.asarray(W_qkv, dtype=np.float32)
    W_o = np.asarray(W_o, dtype=np.float32)
    gamma = np.asarray(gamma, dtype=np.float32)
    beta = np.asarray(beta, dtype=np.float32)

    import ml_dtypes
    bf16 = ml_dtypes.bfloat16
    f8 = ml_dtypes.float8_e4m3fn if USE_FP8 else bf16
    # Permute W_qkv columns from per-head [q|k|v] interleave to [Q|K|V]
    # blocks (V pre-scaled by 1/sqrt(dh)), convert to fp8, partition-major.
    w_blocks = W_qkv.reshape(D, H, 3, DH).transpose(0, 2, 1, 3).copy()
    w_blocks[:, 2] *= SCALE
    w_perm = np.ascontiguousarray(
        w_blocks.reshape(D, F).reshape(KD, P, F).transpose(1, 0, 2)).astype(f8)
    wo_shuf = np.ascontiguousarray(
        W_o.reshape(KD, P, D).transpose(1, 0, 2)).astype(bf16)

    apply_gb = not (np.all(gamma == 1.0) and np.all(beta == 0.0))
    nc = _get_nc(apply_gb)
    in_maps = []
    for b in range(8):
        hb = h[:, b, :]
        in_maps.append({
            "h": np.ascontiguousarray(
                hb.reshape(NLT, P, D).transpose(1, 0, 2)).astype(bf16),
            "hT": np.ascontiguousarray(
                hb.reshape(NLT, P, KD, P).transpose(3, 0, 2, 1)).astype(f8),
            "W_qkv": w_perm,
            "W_o": wo_shuf,
            "gamma": gamma,
            "beta": beta,
        })
    res = run_bass_kernel_spmd(nc, in_maps, core_ids=list(range(8)), trace=trace)
    LAST_RESULT = res
    return np.stack([res.results[b]["out"] for b in range(8)], axis=1)
